# revision 1
# baseline (speedup 1.0000x reference)
"""Trainium2 Bass kernel for nn_ChannelWiseQuantumKernel.

Method: the per-position data RY gates are converted to diagonal phase gates
via RY(t) = (SH) RZ(t) (SH)^dag, so the circuit per patch becomes
    y <- G_pos (e^{i a} . y),   a_i = 0.5 * sum_ch (2 b_ch(i) - 1) theta_ch
with fixed 16x16 complex G_pos precomputed on host from the (tiny) weights.
On device (per core, 2 images of 3844 patches each packed in partitions):
  - alpha via a small tensor-engine matmul (sign matrix x theta)
  - cos/sin via one ScalarE Sin activation with per-partition bias (0 or pi/2)
  - the complex diagonal multiply via ONE VectorE tensor_tensor multiply on an
    extended-state layout [Re; Im; Im; Re] (all 4 real products in one op)
  - G application + re-expansion via one 128x128 tensor-engine matmul
State lives in PSUM ping-pong banks; patches stream in 512-column chunks.
Measurement: Square (ScalarE) + one matmul against a signed-sum matrix.
"""

import sys

sys.path.insert(0, "/opt/trn_rl_repo")

import numpy as np

import concourse.bacc as bacc
import concourse.bass as bass
import concourse.tile as tile
from concourse import mybir
from concourse.bass_utils import run_bass_kernel_spmd

# ---------------- problem constants ----------------
IN_CH = 4
KSZ = 3
NPOS = 9
DIM = 16
B = 16
HW = 64
OH = HW - KSZ + 1  # 62
P = OH * OH  # 3844 patches per image
N_CORES = 8
IMGS_PER_CORE = B // N_CORES  # 2 (the two partition-chunks)
F = P  # free-dim length per chunk (one image per chunk)
CHUNK = 512

# Use float32r (full-rate fp32 path on the PE) for matmuls; toggle for A/B.
MM_DT = mybir.dt.float32r
# bf16 state-stage matmuls: 2x PE clock (HAM warms) + fast weight loads.
STATE_BF16 = False

# wmats column layout (per kernel: B0 | W1..W7 | W8), then SGNL_p (9x), ZL, BIAS
KCOLS = 128 + 7 * 128 + 64  # 1088
COL_SGN = 2 * KCOLS  # 2176; nine (72,128) sign matrices, one per position
COL_ZL = COL_SGN + 9 * 128  # 3328
COL_BIAS = COL_ZL + 8  # 3336
WM_COLS = COL_BIAS + 1  # 3337

# ---------------- host-side constant math (weights-only, O(1)) ----------------
_H = np.array([[1, 1], [1, -1]], dtype=np.complex128) / np.sqrt(2)
_S = np.array([[1, 0], [0, 1j]], dtype=np.complex128)
_A1 = _S @ _H


def _kron_n(mats):
    out = np.array([[1.0 + 0j]])
    for m in mats:
        out = np.kron(out, m)
    return out


_AA = _kron_n([_A1] * IN_CH)
_U1 = _A1.conj().T @ (np.array([1.0, 1.0]) / np.sqrt(2))
_YINIT = _kron_n([_U1.reshape(2, 1)] * IN_CH).reshape(DIM)

_SGN = np.array(
    [[2 * ((i >> (3 - ch)) & 1) - 1 for i in range(DIM)] for ch in range(IN_CH)],
    dtype=np.float64,
)
_SIGMA = np.array(
    [[1 - 2 * ((i >> (3 - q)) & 1) for i in range(DIM)] for q in range(IN_CH)],
    dtype=np.float64,
)


def _rx(t):
    c, s = np.cos(t / 2), np.sin(t / 2)
    return np.array([[c, -1j * s], [-1j * s, c]])


def _ry(t):
    c, s = np.cos(t / 2), np.sin(t / 2)
    return np.array([[c, -s], [s, c]])


def _rz(t):
    e = np.exp(-0.5j * t)
    return np.array([[e, 0], [0, np.conj(e)]])


def _embed(U, q):
    mats = [np.eye(2, dtype=complex)] * IN_CH
    mats[q] = U
    return _kron_n(mats)


def _cx(cq, tq):
    M = np.zeros((DIM, DIM), dtype=complex)
    for i in range(DIM):
        bits = [(i >> (3 - q)) & 1 for q in range(4)]
        j = i
        if bits[cq] == 1:
            bits2 = bits.copy()
            bits2[tq] ^= 1
            j = sum(b << (3 - q) for q, b in enumerate(bits2))
        M[j, i] = 1
    return M


def _build_G(w_flat):
    w = np.float64(w_flat).reshape(NPOS, 1, IN_CH, 3)
    Gs = []
    for pos in range(NPOS):
        U = np.eye(DIM, dtype=complex)
        for q in range(IN_CH):
            R = _rz(w[pos, 0, q, 2]) @ _ry(w[pos, 0, q, 1]) @ _rx(w[pos, 0, q, 0])
            U = _embed(R, q) @ U
        for q in range(IN_CH - 1):
            U = _cx(q, q + 1) @ U
        U = _cx(IN_CH - 1, 0) @ U
        Gs.append(_AA.conj().T @ U @ _AA)
    G8f = _AA @ Gs[8]
    return Gs, G8f


def _wc_of(G, bouts):
    """Extended-state transition block: rows = m blocks [cR, cI, sI, sR],
    cols = output ext blocks listed in `bouts` (0/3 = Re, 1/2 = Im)."""
    Gr, Gi = G.real, G.imag
    Wc = np.zeros((64, 16 * len(bouts)))
    for o, bout in enumerate(bouts):
        re_out = bout in (0, 3)
        for j in range(DIM):
            col = o * 16 + j
            if re_out:
                Wc[0:16, col] = Gr[j]
                Wc[16:32, col] = -Gi[j]
                Wc[32:48, col] = -Gr[j]
                Wc[48:64, col] = -Gi[j]
            else:
                Wc[0:16, col] = Gi[j]
                Wc[16:32, col] = Gr[j]
                Wc[32:48, col] = -Gi[j]
                Wc[48:64, col] = Gr[j]
    return Wc


def _blockdiag2(M):
    Z = np.zeros((128, 2 * M.shape[1]), dtype=np.float32)
    Z[:64, : M.shape[1]] = M
    Z[64:, M.shape[1] :] = M
    return Z


# Triple-angle range reduction: the ScalarE Sin spline is only valid on
# [-pi, pi] but alpha reaches ~5. We compute v = sin(alpha/3 + delta)
# (delta = pi/6 for cos rows, 0 for sin rows; |alpha|/3 + pi/6 < pi), then
# w = (v^2 - 3/4) * v = -Phi/4 , and absorb the -4 into the stage matrices.
_STAGE_SCALE = -4.0


def _build_wmats(weights):
    """(128, WM_COLS) fp32 constant array of all matmul lhsT matrices."""
    wm = np.zeros((128, WM_COLS), dtype=np.float32)
    for k in range(2):
        Gs, G8f = _build_G(weights[k])
        G0c = Gs[0] @ np.diag(_YINIT)
        # pos-0 matrix: rhs is Phi2_0 ([cos;cos;sin;sin] blocks)
        B0c = np.zeros((64, 64))
        G0r, G0i = G0c.real, G0c.imag
        for bout in range(4):
            re_out = bout in (0, 3)
            for j in range(DIM):
                col = bout * 16 + j
                if re_out:
                    B0c[0:16, col] = G0r[j]
                    B0c[32:48, col] = -G0i[j]
                else:
                    B0c[0:16, col] = G0i[j]
                    B0c[32:48, col] = G0r[j]
        base = k * KCOLS
        wm[:, base : base + 128] = _STAGE_SCALE * _blockdiag2(B0c)
        for p in range(1, 8):
            wm[:, base + p * 128 : base + (p + 1) * 128] = _STAGE_SCALE * _blockdiag2(
                _wc_of(Gs[p], [0, 1, 2, 3])
            )
        wm[:, base + 1024 : base + 1088] = _STAGE_SCALE * _blockdiag2(
            _wc_of(G8f, [0, 1])
        )
    # SGNL_p (72, 128) per position: only rows [8p, 8p+8) nonzero.
    # Emits alpha/3 directly (entries +-1/6).
    for pos in range(NPOS):
        for chunk in range(2):
            for ch in range(IN_CH):
                row = np.zeros(128, dtype=np.float32)
                for b in range(4):
                    for i in range(DIM):
                        row[chunk * 64 + b * 16 + i] = _SGN[ch, i] / 6.0
                wm[
                    8 * pos + chunk * 4 + ch,
                    COL_SGN + pos * 128 : COL_SGN + (pos + 1) * 128,
                ] = row
    # ZL (64, 8)
    for chunk in range(2):
        for h in range(2):
            for q in range(IN_CH):
                for i in range(DIM):
                    wm[chunk * 32 + h * 16 + i, COL_ZL + chunk * 4 + q] = _SIGMA[q, i]
    # BIAS (128, 1): pi/6 on cos rows (blocks 0,1), 0 on sin rows (blocks 2,3)
    bias = np.zeros(128, dtype=np.float32)
    for chunk in range(2):
        bias[chunk * 64 : chunk * 64 + 32] = np.pi / 6
    wm[:, COL_BIAS] = bias
    return wm


K16COLS = 7 * 128 + 64  # 960 per kernel


def _build_wmats16(weights):
    """bf16 copies of the state-stage matrices (W1..W7, W8) per kernel."""
    import ml_dtypes

    wm = np.zeros((128, 2 * K16COLS), dtype=ml_dtypes.bfloat16)
    for k in range(2):
        Gs, G8f = _build_G(weights[k])
        base = k * K16COLS
        for p in range(1, 8):
            wm[:, base + (p - 1) * 128 : base + p * 128] = (
                _STAGE_SCALE * _blockdiag2(_wc_of(Gs[p], [0, 1, 2, 3]))
            ).astype(ml_dtypes.bfloat16)
        wm[:, base + 896 : base + 960] = (
            _STAGE_SCALE * _blockdiag2(_wc_of(G8f, [0, 1]))
        ).astype(ml_dtypes.bfloat16)
    return wm


def _build_theta(x):
    """x (16, 4, 64, 64) -> per-core theta arrays (72, F): rows pos*8+chunk*4+ch."""
    xw = np.lib.stride_tricks.sliding_window_view(x, (KSZ, KSZ), axis=(2, 3))
    # (B, C, OH, OW, K, K) -> (B, 9pos, C, P)
    arr = xw.transpose(0, 4, 5, 1, 2, 3).reshape(B, KSZ * KSZ, IN_CH, P)
    out = []
    for c in range(N_CORES):
        t = np.stack([arr[2 * c], arr[2 * c + 1]], axis=1)  # (9, 2, 4, P)
        out.append(np.ascontiguousarray(t.reshape(72, P), dtype=np.float32))
    return out


# ---------------- custom fused DVE op: m = ((v^2 - 3/4) * v) * y ------------
_CUBE_OP = None


def _register_cube_mul():
    """Register the fused triple-angle multiply as a custom DVE op (row 17).

    One VectorE instruction computes ((v*v - c0) * v) * y, which applies the
    full range-reduced phase factor to the state in a single pass."""
    global _CUBE_OP
    if _CUBE_OP is not None:
        return _CUBE_OP
    import concourse.dve_ops as dve_ops

    for o in dve_ops.OPS:
        if o.name == "CUBE_MUL_ANT":
            _CUBE_OP = o
            return o
    from concourse.dve_ops import DveOp
    from concourse.dve_spec import C0, Spec, Src0, Src1, lower
    from concourse.dve_uop import DveOpSpec

    body = ((Src0 * Src0 - C0) * Src0) * Src1
    spec = Spec(
        body=body,
        reference=lambda in0, in1, c0, c1, c2: (
            ((in0.astype(np.float32) * in0 - c0) * in0) * in1
        ),
    )
    row = max(dve_ops._SUB_OPCODE_FOR_NAME.values()) + 1
    shas = {}
    for ver in ("v3", "v4"):
        uops = lower(spec, ver=ver)
        shas[ver] = DveOpSpec(
            name="CUBE_MUL_ANT", opcode=row, uops=uops, rd1_en=True
        ).sha(ver)
    op = DveOp("CUBE_MUL_ANT", spec, subdim=False, uops_sha=shas)
    dve_ops.OPS.append(op)
    dve_ops._SUB_OPCODE_FOR_NAME[op.name] = row
    dve_ops.CUSTOM_DVE_SPECS[op.name] = spec
    _CUBE_OP = op
    return op


# ---------------- device program ----------------
_PROGRAM_CACHE = {}


def _build_program(mm_dt):
    key = str(mm_dt)
    if key in _PROGRAM_CACHE:
        return _PROGRAM_CACHE[key]

    nc = bacc.Bacc("TRN2", target_bir_lowering=False, debug=False)
    th_d = nc.dram_tensor("theta", [72, F], mm_dt, kind="ExternalInput").ap()
    wm_d = nc.dram_tensor(
        "wmats", [128, WM_COLS], mm_dt, kind="ExternalInput"
    ).ap()
    z_d = nc.dram_tensor(
        "zout", [2, 8, F], mybir.dt.float32, kind="ExternalOutput"
    ).ap()
    if STATE_BF16:
        wm16_d = nc.dram_tensor(
            "wmats16", [128, 2 * K16COLS], mybir.dt.bfloat16, kind="ExternalInput"
        ).ap()

    f32 = mybir.dt.float32
    PAIR = 2 * CHUNK
    CUBE = _register_cube_mul()

    chunks = []
    c0 = 0
    while c0 < F:
        chunks.append((c0, min(CHUNK, F - c0)))
        c0 += CHUNK

    with tile.TileContext(nc) as tc:
        from contextlib import ExitStack

        with ExitStack() as ctx:
            const_pool = ctx.enter_context(tc.tile_pool(name="const", bufs=1))
            th_pool = ctx.enter_context(tc.tile_pool(name="th", bufs=4))
            v_pool = ctx.enter_context(tc.tile_pool(name="v", bufs=10))
            w0_pool = ctx.enter_context(tc.tile_pool(name="w0", bufs=3))
            m_pool = ctx.enter_context(tc.tile_pool(name="m", bufs=8))
            sq_pool = ctx.enter_context(tc.tile_pool(name="sq", bufs=2))
            zs_pool = ctx.enter_context(tc.tile_pool(name="zs", bufs=2))
            a_pool = ctx.enter_context(tc.tile_pool(name="aps", bufs=2, space="PSUM"))
            yp = [
                ctx.enter_context(tc.tile_pool(name=f"y{i}", bufs=2, space="PSUM"))
                for i in range(3)
            ]

            wm_sb = const_pool.tile([128, WM_COLS], mm_dt)
            nc.sync.dma_start(wm_sb[:], wm_d[:])
            ones_sb = const_pool.tile([128, CHUNK], f32)
            nc.vector.memset(ones_sb[:], 1.0)

            def sgn_l(p):
                return wm_sb[0:72, COL_SGN + p * 128 : COL_SGN + (p + 1) * 128]

            z_l = wm_sb[0:64, COL_ZL : COL_ZL + 8]
            bias_col = wm_sb[:, COL_BIAS : COL_BIAS + 1].bitcast(f32)

            def w_ap(k, p):
                base = k * KCOLS
                if p == 0:
                    return wm_sb[:, base : base + 128]
                if p < 8:
                    return wm_sb[:, base + p * 128 : base + (p + 1) * 128]
                return wm_sb[:, base + 1024 : base + 1088]

            trig_state = {}
            chain = {}

            def do_dma(ci, _pos):
                c0, C = chunks[ci]
                th_t = th_pool.tile([72, CHUNK], mm_dt, tag="th", name=f"th{ci}")
                nc.sync.dma_start(th_t[:, :C], th_d[:, c0 : c0 + C])
                trig_state[ci] = {"th": th_t, "v": {}, "w0": None}

            def do_trig(ci, pos):
                c0, C = chunks[ci]
                st = trig_state[ci]
                a_ps = a_pool.tile([128, CHUNK], f32, tag="aps", name=f"a{ci}_{pos}")
                nc.tensor.matmul(
                    a_ps[:, :C], sgn_l(pos), st["th"][0:72, :C], start=True, stop=True
                )
                v_t = v_pool.tile([128, CHUNK], f32, tag="v", name=f"v{ci}_{pos}")
                nc.scalar.activation(
                    v_t[:, :C],
                    a_ps[:, :C],
                    mybir.ActivationFunctionType.Sin,
                    bias=bias_col,
                    scale=1.0,
                )
                st["v"][pos] = v_t
                if pos == 0:
                    w0 = w0_pool.tile([128, CHUNK], mm_dt, tag="w0", name=f"w0_{ci}")
                    nc.vector._custom_dve(
                        CUBE,
                        out=w0[:, :C],
                        in0=v_t[:, :C],
                        in1=ones_sb[:, :C],
                        s0=0.75,
                    )
                    st["w0"] = w0

            def do_state(ci, pos):
                c0, C = chunks[ci]
                st = trig_state[ci]
                cur = chain.setdefault(ci, [None, None])
                for k in range(2):
                    pool = yp[(2 * ci + k) % 3]
                    y_new = pool.tile(
                        [128, CHUNK],
                        f32,
                        tag=f"y{(2 * ci + k) % 3}",
                        name=f"y{ci}_{k}_{pos}",
                    )
                    if pos == 0:
                        nc.tensor.matmul(
                            y_new[:, :C],
                            w_ap(k, 0),
                            st["w0"][:, :C],
                            start=True,
                            stop=True,
                        )
                    else:
                        m_t = m_pool.tile([128, CHUNK], mm_dt, tag="m")
                        nc.vector._custom_dve(
                            CUBE,
                            out=m_t[:, :C],
                            in0=st["v"][pos][:, :C],
                            in1=cur[k][:, :C],
                            s0=0.75,
                        )
                        nc.tensor.matmul(
                            y_new[0:64, :C] if pos == 8 else y_new[:, :C],
                            w_ap(k, pos),
                            m_t[:, :C],
                            start=True,
                            stop=True,
                        )
                    cur[k] = y_new

            def do_meas(ci, _pos):
                c0, C = chunks[ci]
                cur = chain.pop(ci)
                trig_state.pop(ci)
                sq = sq_pool.tile([64, PAIR], mm_dt, tag="sq")
                zs = zs_pool.tile([8, PAIR], f32, tag="zs")
                for k in range(2):
                    nc.scalar.activation(
                        sq[0:64, k * CHUNK : k * CHUNK + C],
                        cur[k][0:64, :C],
                        mybir.ActivationFunctionType.Square,
                    )
                    pool = yp[(2 * ci + k) % 3]
                    zq = pool.tile(
                        [128, CHUNK],
                        f32,
                        tag=f"y{(2 * ci + k) % 3}",
                        name=f"zq{ci}_{k}",
                    )
                    nc.tensor.matmul(
                        zq[0:8, :C],
                        z_l,
                        sq[0:64, k * CHUNK : k * CHUNK + C],
                        start=True,
                        stop=True,
                    )
                    nc.scalar.activation(
                        zs[0:8, k * CHUNK : k * CHUNK + C],
                        zq[0:8, :C],
                        mybir.ActivationFunctionType.Copy,
                    )
                    nc.sync.dma_start(
                        z_d[k, :, c0 : c0 + C],
                        zs[0:8, k * CHUNK : k * CHUNK + C],
                    )

            # merged time-ordered emission: two state chains in flight (offset
            # 4.5 positions), trig running two chunks ahead of its state chain
            OFF = 2.0
            LEAD = 3
            events = []
            nch = len(chunks)
            for ci in range(nch):
                t0 = ci * OFF
                events.append((t0 - LEAD - 0.5, 0, ci, 0, do_dma))
                for pos in range(NPOS):
                    events.append((t0 - LEAD + pos, 1, ci, pos, do_trig))
                    events.append((t0 + pos, 2, ci, pos, do_state))
                events.append((t0 + NPOS, 3, ci, 0, do_meas))
            events.sort(key=lambda e: (e[0], e[1], e[2]))
            for _t, _kind, ci, pos, fn in events:
                fn(ci, pos)

    nc.compile()
    _PROGRAM_CACHE[key] = nc
    return nc


# ---------------- entry point ----------------
def kernel(x, weights):
    x = np.asarray(x, dtype=np.float32)
    weights = np.asarray(weights, dtype=np.float32)
    wm = _build_wmats(weights)
    thetas = _build_theta(x)

    nc = _build_program(MM_DT)
    in_maps = [{"theta": thetas[c], "wmats": wm} for c in range(N_CORES)]
    if STATE_BF16:
        wm16 = _build_wmats16(weights)
        for im in in_maps:
            im["wmats16"] = wm16
    res = run_bass_kernel_spmd(nc, in_maps, list(range(N_CORES)))

    out = np.zeros((B, 2 * IN_CH, OH, OH), dtype=np.float32)
    for c in range(N_CORES):
        z = np.asarray(res.results[c]["zout"])  # (2, 8, F)
        for k in range(2):
            for chunk in range(2):
                b = 2 * c + chunk
                out[b, k * 4 : k * 4 + 4] = z[k, chunk * 4 : chunk * 4 + 4].reshape(
                    IN_CH, OH, OH
                )
    return out

